# revision 1
# baseline (speedup 1.0000x reference)
"""Trainium2 Bass kernel for nn_ModelMultitaskBinary (MMoE multitask binary loss).

Strategy: data-parallel over batch B=512 across 8 cores (64 samples -> 1920
candidate rows per core). All params replicated. No collectives: each core
emits its 64 per-sample losses; the host averages 512 values.

On-chip pipeline per core (activations feature-major [feat(part), row(free)],
matmul inputs bf16, accumulation fp32 in PSUM):
  xT -> h1 = relu(fc1) -> h = fc2 -> glog (row-major via lhsT=h slices)
  -> top-3-of-6 gating (DVE, batched) -> experts in row-group blocks:
  ehT (feature-major), eo (row-major, staged to SBUF bf16),
  y_t += gate*eo on DVE (bf16 perf modes) -> per task: PE-transpose y,
  tower1, logits (row-major) -> BCE + aux load-balance loss -> [64] losses.

Two program variants: zero_bias (all bias inputs are zeros, per the spec
fills: skips bias adds / bias matmuls) and the general fallback.
"""
import os
import sys
from contextlib import ExitStack

for _p in ("/opt/trn_rl_repo", "/root/.axon_site/_ro/trn_rl_repo"):
    if os.path.isdir(_p) and _p not in sys.path:
        sys.path.insert(0, _p)

import numpy as np
import ml_dtypes

import concourse.bass as bass
import concourse.tile as tile
from concourse import bacc, mybir
from concourse.masks import make_identity
from concourse.bass_utils import run_bass_kernel_spmd

F32 = mybir.dt.float32
BF16 = mybir.dt.bfloat16
BF = ml_dtypes.bfloat16
AF = mybir.ActivationFunctionType
OP = mybir.AluOpType
AX = mybir.AxisListType

NCORES = 8
B, C, T, H, E, EH, TH = 512, 30, 3, 512, 6, 512, 512
BSH = B // NCORES          # 64 samples per core
R = BSH * C                # 1920 rows per core
NRT = R // 128             # 15 row tiles
KC = H // 128              # 4 feature chunks
RS = [(0, 512), (512, 1024), (1024, 1536), (1536, R)]  # row slices (<=512)
RG_RT = [(0, 4), (4, 8), (8, 12), (12, 15)]            # row tiles per group
LOSS_COEF = 0.01

# engine-assignment knobs (tuned against the timeline cost model)
KNOBS = {
    "eo_copy_dve_every": 2,   # every k-th eo psum->sbuf copy goes to DVE
    "ts_pool_every": 4,       # every k-th gate-scale mult on GpSimd
    "add_pool_every": 5,      # every k-th y-accumulate add goes to GpSimd
    "ytr_copy_act_every": 2,  # every k-th y-transpose psum->sbuf copy on ACT
}

_CACHED = {}


def build_nc(zero_bias: bool):
    nc = bacc.Bacc(None, target_bir_lowering=False, debug=False)

    xT_d = nc.dram_tensor("xT", [KC, 128, R], BF16, kind="ExternalInput")
    scores_d = nc.dram_tensor("scores", [BSH, T, C], F32, kind="ExternalInput")
    fc1w_d = nc.dram_tensor("fc1w", [KC, 128, H], BF16, kind="ExternalInput")
    fc1b_d = nc.dram_tensor("fc1b", [128, KC], F32, kind="ExternalInput")
    fc2w_d = nc.dram_tensor("fc2w", [KC, 128, H], BF16, kind="ExternalInput")
    fc2b_d = nc.dram_tensor("fc2b", [128, KC], F32, kind="ExternalInput")
    wg_d = nc.dram_tensor("wg", [KC, 128, T * E], BF16, kind="ExternalInput")
    ew1_d = nc.dram_tensor("ew1", [E, KC, 128, EH], BF16, kind="ExternalInput")
    eb1_d = nc.dram_tensor("eb1", [E, 128, KC], F32, kind="ExternalInput")
    ew2_d = nc.dram_tensor("ew2", [E, KC, 128, H], BF16, kind="ExternalInput")
    eb2_d = nc.dram_tensor("eb2", [E, 1, H], BF16, kind="ExternalInput")
    tw1_d = nc.dram_tensor("tw1", [T, KC, 128, TH], BF16, kind="ExternalInput")
    tb1_d = nc.dram_tensor("tb1", [T, 128, KC], F32, kind="ExternalInput")
    tw2_d = nc.dram_tensor("tw2", [T, 128, KC], BF16, kind="ExternalInput")
    tb2_d = nc.dram_tensor("tb2", [128, T], F32, kind="ExternalInput")
    sel_d = nc.dram_tensor("sel", [NRT, 128, BSH], F32, kind="ExternalInput")
    selt_d = nc.dram_tensor("selt", [NRT, BSH, 128], F32, kind="ExternalInput")
    srm_d = nc.dram_tensor("srm", [128, NRT, T], F32, kind="ExternalInput")
    loss_d = nc.dram_tensor("loss", [BSH, 1], F32, kind="ExternalOutput")

    eo_dve = KNOBS["eo_copy_dve_every"]
    ts_pool = KNOBS.get("ts_pool_every", 5)
    add_pool = KNOBS["add_pool_every"]
    ytr_act = KNOBS["ytr_copy_act_every"]

    with tile.TileContext(nc, pool_alloc_mode="queue") as tc, ExitStack() as ctx:
        perm = ctx.enter_context(tc.tile_pool(name="perm", bufs=1))
        dram = ctx.enter_context(tc.tile_pool(name="dram", bufs=1, space="DRAM"))
        psA = ctx.enter_context(tc.tile_pool(name="psA", bufs=5, space="PSUM"))
        psB = ctx.enter_context(tc.tile_pool(name="psB", bufs=2, space="PSUM"))
        hpool = ctx.enter_context(tc.tile_pool(name="hpool", bufs=1))

        ident_bf = perm.tile([128, 128], BF16)
        make_identity(nc, ident_bf)
        ident_f = perm.tile([128, 128], F32)
        make_identity(nc, ident_f)
        if not zero_bias:
            ones_bf = perm.tile([1, 128], BF16)
            nc.vector.memset(ones_bf, 1.0)

        warm = perm.tile([128, 1], F32)
        nc.scalar.activation(warm, ident_f[:, 0:1], AF.Exp)
        nc.scalar.activation(warm, ident_f[:, 0:1], AF.Abs)
        nc.scalar.activation(warm, ident_f[:, 0:1], AF.Ln, bias=1.0)

        scores_sb = perm.tile([BSH, T, C], F32)
        nc.sync.dma_start(scores_sb, scores_d[:, :, :])
        srm_sb = perm.tile([128, NRT, T], F32)
        sel_sb = [perm.tile([128, BSH], F32, name=f"sel{rt}") for rt in range(NRT)]
        selt_sb = [perm.tile([BSH, 128], F32, name=f"selt{rt}") for rt in range(NRT)]
        if not zero_bias:
            tb2_sb = perm.tile([128, 1, T], F32)
            nc.sync.dma_start(tb2_sb, tb2_d[:, :])

        glog = perm.tile([128, NRT * T * E], F32)    # [128, 270] row-major
        gates = perm.tile([128, NRT * T * E], F32)
        gates_fm = perm.tile([T * E, R], F32)        # [18, 1920] feature-major
        ypool = ctx.enter_context(tc.tile_pool(name="ypool", bufs=1))
        yT = [ypool.tile([128, KC * R], BF16, name=f"yT{t}") for t in range(T)]
        logits_sb = perm.tile([128, NRT, T], F32)

        hT = [hpool.tile([128, R], BF16, name=f"hT{k}") for k in range(KC)]

        # expert weights: resident for the whole expert phase
        epool = ctx.enter_context(tc.tile_pool(name="epool", bufs=1))

        # ---------------- phase 1+2: shared bottom ----------------
        with tc.tile_pool(name="early", bufs=1) as early:
            fc1w = [early.tile([128, H], BF16, name=f"fc1w{k}")
                    for k in range(KC)]
            fc2w = [early.tile([128, H], BF16, name=f"fc2w{k}")
                    for k in range(KC)]
            wgw = [early.tile([128, T * E], BF16, name=f"wg{k}")
                   for k in range(KC)]
            xT = [early.tile([128, R], BF16, name=f"xT{k}") for k in range(KC)]
            r0, r1 = RS[0]
            for k in range(KC):
                nc.sync.dma_start(fc1w[k], fc1w_d[k, :, :])
                nc.sync.dma_start(xT[k][:, r0:r1], xT_d[k, :, r0:r1])
            for k in range(KC):
                nc.sync.dma_start(wgw[k], wg_d[k, :, :])
                nc.sync.dma_start(fc2w[k], fc2w_d[k, :, :])
            for (r0, r1) in RS[1:]:
                for k in range(KC):
                    nc.sync.dma_start(xT[k][:, r0:r1], xT_d[k, :, r0:r1])
            if not zero_bias:
                fc1b = early.tile([128, KC], F32)
                nc.sync.dma_start(fc1b, fc1b_d[:, :])
                fc2b = early.tile([128, KC], F32)
                nc.sync.dma_start(fc2b, fc2b_d[:, :])

            ew1 = [[None] * KC for _ in range(E)]
            ew2 = [[None] * KC for _ in range(E)]
            eb1 = [None] * E
            eb2row = [None] * E
            for e in range(E):
                for k in range(KC):
                    w1 = epool.tile([128, EH], BF16, name=f"ew1_{e}_{k}")
                    nc.sync.dma_start(w1, ew1_d[e, k, :, :])
                    ew1[e][k] = w1
                    w2 = epool.tile([128, H], BF16, name=f"ew2_{e}_{k}")
                    nc.sync.dma_start(w2, ew2_d[e, k, :, :])
                    ew2[e][k] = w2
                if not zero_bias:
                    b1 = epool.tile([128, KC], F32, name=f"eb1_{e}")
                    nc.sync.dma_start(b1, eb1_d[e, :, :])
                    eb1[e] = b1
                    b2r = epool.tile([1, H], BF16, name=f"eb2_{e}")
                    nc.sync.dma_start(b2r, eb2_d[e, :, :])
                    eb2row[e] = b2r

            nc.sync.dma_start(srm_sb, srm_d[:, :, :])
            for rt in range(NRT):
                nc.sync.dma_start(sel_sb[rt], sel_d[rt, :, :])
                nc.sync.dma_start(selt_sb[rt], selt_d[rt, :, :])

            h1T = [early.tile([128, R], BF16, name=f"h1T{k}") for k in range(KC)]
            for mc in range(KC):
                for (r0, r1) in RS:
                    ps = psA.tile([128, r1 - r0], F32, name="accB", tag="acc")
                    for k in range(KC):
                        nc.tensor.matmul(
                            ps, fc1w[k][:, mc * 128:(mc + 1) * 128], xT[k][:, r0:r1],
                            start=(k == 0), stop=(k == KC - 1))
                    if zero_bias:
                        nc.scalar.activation(h1T[mc][:, r0:r1], ps, AF.Relu)
                    else:
                        nc.scalar.activation(h1T[mc][:, r0:r1], ps, AF.Relu,
                                             bias=fc1b[:, mc:mc + 1])
            for mc in range(KC):
                for (r0, r1) in RS:
                    ps = psA.tile([128, r1 - r0], F32, name="accB2", tag="acc")
                    for k in range(KC):
                        nc.tensor.matmul(
                            ps, fc2w[k][:, mc * 128:(mc + 1) * 128], h1T[k][:, r0:r1],
                            start=(k == 0), stop=(k == KC - 1))
                    if zero_bias:
                        nc.scalar.activation(hT[mc][:, r0:r1], ps, AF.Copy)
                    else:
                        nc.scalar.activation(hT[mc][:, r0:r1], ps, AF.Identity,
                                             bias=fc2b[:, mc:mc + 1])

            # ---------------- phase 3: gate logits (row-major) ----------------
            GE = T * E
            for rt in range(NRT):
                ps = psA.tile([128, GE], F32, name="accG", tag="acc")
                for k in range(KC):
                    nc.tensor.matmul(
                        ps, hT[k][:, rt * 128:(rt + 1) * 128], wgw[k],
                        start=(k == 0), stop=(k == KC - 1))
                nc.scalar.activation(glog[:, rt * GE:(rt + 1) * GE], ps, AF.Copy)

        # ---------------- gating: top-3-of-6 masked softmax ----------------
        NG = NRT * T  # 45 groups of E
        v = glog.rearrange("p (g e) -> p g e", e=E)
        gtmp = ctx.enter_context(tc.tile_pool(name="gtmp", bufs=1))  # noqa
        neginf = gtmp.tile([128, NG, E], F32)
        nc.vector.memset(neginf, -1e30)
        m1 = gtmp.tile([128, NG, 1], F32)
        nc.vector.tensor_reduce(m1, v, AX.X, OP.max)
        m1b = m1.broadcast_to([128, NG, E])
        mask = gtmp.tile([128, NG, E], mybir.dt.uint8)
        nc.vector.tensor_tensor(mask, v, m1b, OP.is_ge)
        v2 = gtmp.tile([128, NG, E], F32)
        nc.vector.select(v2, mask, neginf, v)
        m2 = gtmp.tile([128, NG, 1], F32)
        nc.vector.tensor_reduce(m2, v2, AX.X, OP.max)
        mask2 = gtmp.tile([128, NG, E], mybir.dt.uint8)
        nc.vector.tensor_tensor(mask2, v2, m2.broadcast_to([128, NG, E]), OP.is_ge)
        v3 = gtmp.tile([128, NG, E], F32)
        nc.vector.select(v3, mask2, neginf, v2)
        m3 = gtmp.tile([128, NG, 1], F32)
        nc.vector.tensor_reduce(m3, v3, AX.X, OP.max)
        keep = gtmp.tile([128, NG, E], F32)
        nc.vector.tensor_tensor(keep, v, m3.broadcast_to([128, NG, E]), OP.is_ge)
        vs = gtmp.tile([128, NG, E], F32)
        nc.vector.tensor_tensor(vs, v, m1b, OP.subtract)
        ex = gtmp.tile([128, NG, E], F32)
        nc.scalar.activation(ex, vs, AF.Exp)
        ek = gtmp.tile([128, NG, E], F32)
        nc.vector.tensor_tensor(ek, ex, keep, OP.mult)
        ssum = gtmp.tile([128, NG, 1], F32)
        nc.vector.tensor_reduce(ssum, ek, AX.X, OP.add)
        rsum = gtmp.tile([128, NG, 1], F32)
        nc.vector.reciprocal(rsum, ssum)
        gv = gates.rearrange("p (g e) -> p g e", e=E)
        nc.vector.tensor_tensor(gv, ek, rsum.broadcast_to([128, NG, E]), OP.mult)

        # gates feature-major (for aux loss): PE transpose per row tile
        GE = T * E
        for rt in range(NRT):
            gp = psB.tile([GE, 128], F32, name="gtr", tag="small", bufs=1)
            nc.tensor.transpose(gp, gates[:, rt * GE:(rt + 1) * GE], ident_f)
            nc.vector.tensor_copy(gates_fm[:, rt * 128:(rt + 1) * 128], gp)

        # aux: imp[t,e,b] = sum_c gates_fm -> cv^2 per (b,t)
        imp = perm.tile([T * E, BSH], F32)
        nc.vector.tensor_reduce(
            imp, gates_fm.rearrange("p (b c) -> p b c", c=C), AX.X, OP.add)
        ip = psB.tile([BSH, T * E], F32, name="itr", tag="small", bufs=1)
        nc.tensor.transpose(ip, imp, ident_f[:T * E, :T * E])
        impT = perm.tile([BSH, T * E], F32)
        nc.vector.tensor_copy(impT, ip)
        impTv = impT.rearrange("b (t e) -> b t e", e=E)
        auxs = perm.tile([BSH, 1], F32)
        for t in range(T):
            st = perm.tile([BSH, 6], F32, name=f"bnst{t}")
            nc.vector.bn_stats(st, impTv[:, t, :])
            mv = perm.tile([BSH, 2], F32, name=f"bnmv{t}")
            nc.vector.bn_aggr(mv, st)
            msq = perm.tile([BSH, 1], F32, name=f"msq{t}")
            nc.vector.tensor_tensor(msq, mv[:, 0:1], mv[:, 0:1], OP.mult)
            nc.vector.tensor_scalar(msq, msq, 1e-10, None, OP.add)
            rec = perm.tile([BSH, 1], F32, name=f"rec{t}")
            nc.vector.reciprocal(rec, msq)
            cv2 = perm.tile([BSH, 1], F32, name=f"cv2{t}")
            nc.vector.tensor_tensor(cv2, mv[:, 1:2], rec, OP.mult)
            if t == 0:
                nc.vector.tensor_copy(auxs, cv2)
            else:
                nc.vector.tensor_tensor(auxs, auxs, cv2, OP.add)

        # ------------- phase 4: experts, row-group blocked -------------
        nco = 0  # rotating index for engine-split knobs
        _st = {"n": 0}

        def emit_transposes(rg):
            pt0, pt1, pyg = rg
            for rtl in range(pt1 - pt0):
                rt = pt0 + rtl
                for t in range(T):
                    tp = psB.tile([128, KC, 128], BF16, name="ytr", tag="tr",
                                  bufs=2)
                    for jc in range(KC):
                        nc.tensor.transpose(
                            tp[:, jc, :], pyg[t][rtl][:, jc * 128:(jc + 1) * 128],
                            ident_bf)
                    dst = bass.AP(
                        tensor=yT[t].tensor, offset=yT[t].offset + rt * 128,
                        ap=[yT[t].ap[0], [R, KC], [1, 128]])
                    _st["n"] += 1
                    if _st["n"] % ytr_act == 0:
                        nc.scalar.activation(dst, tp, AF.Copy)
                    else:
                        nc.vector.tensor_copy(dst, tp)

        prev_rg = None
        with tc.tile_pool(name="exp", bufs=2) as exp:
            for gi, ((r0, r1), (t0, t1)) in enumerate(zip(RS, RG_RT)):
                rgw = r1 - r0
                yg = [[exp.tile([128, H], BF16, name=f"yg{t}_{rtl}", tag="yg",
                                bufs=20) for rtl in range(t1 - t0)]
                      for t in range(T)]
                for e in range(E):
                    if e == 2 and prev_rg is not None:
                        emit_transposes(prev_rg)
                        prev_rg = None
                    ehs = [exp.tile([128, rgw], BF16, name=f"ehs{k}",
                                    tag=f"ehs{k}") for k in range(KC)]
                    for mc in range(KC):
                        ps = psA.tile([128, rgw], F32, name="accE", tag="acc")
                        for k in range(KC):
                            nc.tensor.matmul(
                                ps, ew1[e][k][:, mc * 128:(mc + 1) * 128],
                                hT[k][:, r0:r1],
                                start=(k == 0), stop=(k == KC - 1))
                        if zero_bias:
                            nc.scalar.activation(ehs[mc], ps, AF.Relu)
                        else:
                            nc.scalar.activation(ehs[mc], ps, AF.Relu,
                                                 bias=eb1[e][:, mc:mc + 1])
                    for rtl in range(t1 - t0):
                        rt = t0 + rtl
                        ps = psA.tile([128, H], F32, name="accO", tag="acc")
                        for k in range(KC):
                            nc.tensor.matmul(
                                ps, ehs[k][:, rtl * 128:(rtl + 1) * 128], ew2[e][k],
                                start=(k == 0),
                                stop=(k == KC - 1) and zero_bias)
                        if not zero_bias:
                            nc.tensor.matmul(ps, ones_bf, eb2row[e],
                                             start=False, stop=True)
                        # evacuate eo once; combine from SBUF bf16 (fast modes)
                        eo = exp.tile([128, H], BF16, name="eo", tag="eo",
                                      bufs=6)
                        nco += 1
                        if (nco % eo_dve == 0) if eo_dve > 0 else (nco % -eo_dve != 0):
                            nc.vector.tensor_copy(eo, ps)
                        else:
                            nc.scalar.activation(eo, ps, AF.Copy)
                        for t in range(T):
                            g_ap = gates[:, rt * 18 + t * 6 + e:
                                         rt * 18 + t * 6 + e + 1]
                            nco += 1
                            if e == 0:
                                nc.vector.tensor_scalar(
                                    yg[t][rtl], eo, g_ap, None, OP.mult)
                            else:
                                tmp = exp.tile([128, H], BF16, name="ysc",
                                               tag="ysc", bufs=4)
                                if nco % ts_pool == 0:
                                    nc.gpsimd.tensor_scalar(tmp, eo, g_ap,
                                                            None, OP.mult)
                                else:
                                    nc.vector.tensor_scalar(tmp, eo, g_ap,
                                                            None, OP.mult)
                                if nco % add_pool == 0:
                                    nc.gpsimd.tensor_tensor(
                                        yg[t][rtl], yg[t][rtl], tmp, OP.add)
                                else:
                                    nc.vector.tensor_tensor(
                                        yg[t][rtl], yg[t][rtl], tmp, OP.add)
                prev_rg = (t0, t1, yg)
            emit_transposes(prev_rg)

        # labels in row-major layout: smax -> broadcast (selector matmuls)
        smax = perm.tile([BSH, T], F32)
        smax3 = perm.tile([BSH, T, 1], F32)
        nc.vector.tensor_reduce(smax3, scores_sb, AX.X, OP.max)
        nc.vector.tensor_copy(smax, smax3.rearrange("b t one -> b (t one)"))
        smax_bc = perm.tile([128, NRT, T], F32)
        for rt in range(NRT):
            pb = psB.tile([128, T], F32, name="smb", tag="small", bufs=1)
            nc.tensor.matmul(pb, selt_sb[rt], smax, start=True, stop=True)
            nc.vector.tensor_copy(smax_bc[:, rt, :], pb)
        labels_rm = perm.tile([128, NRT, T], F32)
        nc.vector.tensor_tensor(labels_rm, srm_sb, smax_bc, OP.is_equal)

        # ---------------- phase 5: towers ----------------
        with tc.tile_pool(name="tow", bufs=2) as tow:
            for t in range(T):
                tw1 = []
                for k in range(KC):
                    w1 = tow.tile([128, TH], BF16, name=f"tw1_{k}", tag=f"tw1_{k}")
                    nc.sync.dma_start(w1, tw1_d[t, k, :, :])
                    tw1.append(w1)
                if not zero_bias:
                    tb1 = tow.tile([128, KC], F32, tag="tb1")
                    nc.sync.dma_start(tb1, tb1_d[t, :, :])
                tw2 = tow.tile([128, KC], BF16, tag="tw2")
                nc.sync.dma_start(tw2, tw2_d[t, :, :])

                thT = [tow.tile([128, R], BF16, name=f"thT{k}", tag=f"thT{k}", bufs=1)
                       for k in range(KC)]
                for mc in range(KC):
                    for (r0, r1) in RS:
                        ps = psA.tile([128, r1 - r0], F32, name="accT", tag="acc")
                        for k in range(KC):
                            nc.tensor.matmul(
                                ps, tw1[k][:, mc * 128:(mc + 1) * 128], yT[t][:, k * R + r0:k * R + r1],
                                start=(k == 0), stop=(k == KC - 1))
                        if zero_bias:
                            nc.scalar.activation(thT[mc][:, r0:r1], ps, AF.Relu)
                        else:
                            nc.scalar.activation(thT[mc][:, r0:r1], ps, AF.Relu,
                                                 bias=tb1[:, mc:mc + 1])
                for rt in range(NRT):
                    pl = psB.tile([128, 1], F32, name="lg", tag="small", bufs=1)
                    for k in range(KC):
                        nc.tensor.matmul(
                            pl, thT[k][:, rt * 128:(rt + 1) * 128], tw2[:, k:k + 1],
                            start=(k == 0), stop=(k == KC - 1))
                    nc.vector.tensor_copy(logits_sb[:, rt, t:t + 1], pl)

        # ---------------- phase 6: BCE (row-major) ----------------
        lg = logits_sb  # [128, NRT, T]
        if not zero_bias:
            nc.vector.tensor_tensor(lg, lg, tb2_sb.broadcast_to([128, NRT, T]),
                                    OP.add)
        t1_ = perm.tile([128, NRT, T], F32)
        nc.vector.tensor_scalar(t1_, lg, 0.0, None, OP.max)
        t2_ = perm.tile([128, NRT, T], F32)
        nc.vector.tensor_tensor(t2_, lg, labels_rm, OP.mult)
        absl = perm.tile([128, NRT, T], F32)
        nc.scalar.activation(absl, lg, AF.Abs)
        expl = perm.tile([128, NRT, T], F32)
        nc.scalar.activation(expl, absl, AF.Exp, scale=-1.0)
        lp = perm.tile([128, NRT, T], F32)
        nc.scalar.activation(lp, expl, AF.Ln, bias=1.0)
        nc.vector.tensor_tensor(t1_, t1_, t2_, OP.subtract)
        nc.vector.tensor_tensor(t1_, t1_, lp, OP.add)
        bs = perm.tile([128, NRT], F32)
        nc.vector.tensor_reduce(bs, t1_, AX.X, OP.add)
        pb = psB.tile([BSH, 1], F32, name="bsum", tag="small", bufs=1)
        for rt in range(NRT):
            nc.tensor.matmul(pb, sel_sb[rt], bs[:, rt:rt + 1],
                             start=(rt == 0), stop=(rt == NRT - 1))
        tsum = perm.tile([BSH, 1], F32)
        nc.vector.tensor_copy(tsum, pb)

        loss_sb = perm.tile([BSH, 1], F32)
        nc.vector.tensor_scalar(loss_sb, tsum, 1.0 / (T * C), None, OP.mult)
        auxf = perm.tile([BSH, 1], F32)
        nc.vector.tensor_scalar(auxf, auxs, LOSS_COEF, None, OP.mult)
        nc.vector.tensor_tensor(loss_sb, loss_sb, auxf, OP.add)
        nc.sync.dma_start(loss_d[:, :], loss_sb)

    nc.compile()
    return nc


def get_nc(zero_bias=True):
    key = (zero_bias, tuple(sorted(KNOBS.items())))
    if key not in _CACHED:
        _CACHED[key] = build_nc(zero_bias)
    return _CACHED[key]



_SEL_CACHE = None


def _sel_mats():
    """0/1 selector matrices mapping rows r=rt*128+p to samples b=r//30."""
    global _SEL_CACHE
    if _SEL_CACHE is None:
        sel = np.zeros((NRT, 128, BSH), np.float32)
        for rt in range(NRT):
            for p in range(128):
                b = (rt * 128 + p) // C
                sel[rt, p, b] = 1.0
        selt = np.ascontiguousarray(sel.transpose(0, 2, 1))
        _SEL_CACHE = (sel, selt)
    return _SEL_CACHE


def host_prep(inputs):
    """Shard + cast + rearrange the full inputs into 8 per-core in_maps."""
    x = np.asarray(inputs["candidate_cls_embed"], np.float32)
    scores = np.asarray(inputs["scores"], np.float32)
    fc1_w = np.asarray(inputs["fc1_w"], np.float32)
    fc1_b = np.asarray(inputs["fc1_b"], np.float32)
    fc2_w = np.asarray(inputs["fc2_w"], np.float32)
    fc2_b = np.asarray(inputs["fc2_b"], np.float32)
    w_gate = np.asarray(inputs["w_gate"], np.float32)
    expert_w1 = np.asarray(inputs["expert_w1"], np.float32)
    expert_b1 = np.asarray(inputs["expert_b1"], np.float32)
    expert_w2 = np.asarray(inputs["expert_w2"], np.float32)
    expert_b2 = np.asarray(inputs["expert_b2"], np.float32)
    tower_w1 = np.asarray(inputs["tower_w1"], np.float32)
    tower_b1 = np.asarray(inputs["tower_b1"], np.float32)
    tower_w2 = np.asarray(inputs["tower_w2"], np.float32)
    tower_b2 = np.asarray(inputs["tower_b2"], np.float32)

    zero_bias = not (fc1_b.any() or fc2_b.any() or expert_b1.any()
                     or expert_b2.any() or tower_b1.any() or tower_b2.any())

    shared = {
        "fc1w": fc1_w.astype(BF).reshape(KC, 128, H),
        "fc1b": np.ascontiguousarray(fc1_b.reshape(KC, 128).T),
        "fc2w": fc2_w.astype(BF).reshape(KC, 128, H),
        "fc2b": np.ascontiguousarray(fc2_b.reshape(KC, 128).T),
        "wg": np.ascontiguousarray(w_gate.transpose(1, 0, 2).reshape(H, T * E))
              .astype(BF).reshape(KC, 128, T * E),
        "ew1": expert_w1.astype(BF).reshape(E, KC, 128, EH),
        "eb1": np.ascontiguousarray(
            expert_b1.reshape(E, KC, 128).transpose(0, 2, 1)),
        "ew2": expert_w2.astype(BF).reshape(E, KC, 128, H),
        "eb2": expert_b2.astype(BF).reshape(E, 1, H),
        "tw1": tower_w1.astype(BF).reshape(T, KC, 128, TH),
        "tb1": np.ascontiguousarray(
            tower_b1.reshape(T, KC, 128).transpose(0, 2, 1)),
        "tw2": np.ascontiguousarray(
            tower_w2.reshape(T, KC, 128).transpose(0, 2, 1)).astype(BF),
        "tb2": np.ascontiguousarray(
            np.broadcast_to(tower_b2[None, :], (128, T))),
        "sel": _sel_mats()[0],
        "selt": _sel_mats()[1],
    }
    in_maps = []
    for ci in range(NCORES):
        xs = x[ci * BSH:(ci + 1) * BSH].reshape(R, H)
        xT = np.ascontiguousarray(xs.T).astype(BF).reshape(KC, 128, R)
        m = dict(shared)
        m["xT"] = xT
        sc = np.ascontiguousarray(scores[ci * BSH:(ci + 1) * BSH])
        m["scores"] = sc
        srm = sc.transpose(0, 2, 1).reshape(NRT, 128, T).transpose(1, 0, 2)
        m["srm"] = np.ascontiguousarray(srm)
        in_maps.append(m)
    return in_maps, zero_bias


def kernel(**inputs) -> np.ndarray:
    in_maps, zero_bias = host_prep(inputs)
    nc = get_nc(zero_bias)
    res = run_bass_kernel_spmd(nc, in_maps, list(range(NCORES)))
    losses = np.concatenate([res.results[i]["loss"].reshape(-1)
                             for i in range(NCORES)])
    return np.float32(losses.mean(dtype=np.float64))



# revision 37
# speedup vs baseline: 2.0212x; 2.0212x over previous
"""Trainium2 Bass kernel for nn_ModelMultitaskBinary (MMoE multitask binary loss).

Strategy: data-parallel over batch B=512 across 8 cores (64 samples -> 1920
candidate rows per core). All params replicated. No collectives: each core
emits its 64 per-sample losses; the host averages 512 values.

v2: fp8(e4m3) DoubleRow matmuls (0.5 cyc/row) for fc1/fc2/experts/towers with
power-of-2 scale management (weights x16, activations rescaled at each PSUM
evacuation). The per-task gated combine y_t = sum_e g_te*eo_e is fused into
PE matmuls: yT[mc](t,rows) += eo_pair^T @ wide_diag(g) -- which also emits y
feature-major (transpose folded in) and accumulates over experts in PSUM.
Wide diag(g) tiles are built as ident*g (per-partition scalar). The aux-loss
importance matrix is computed by selector matmuls (no transposes). PSUM
evacuations are load-balanced across ACT/DVE/Pool via knobs.

Scales (zero-bias): weights x16; h1T8=4*h1, hT8=4*h, ehs8=8*eh, eo8=16*eo,
g8=fp8(gates), yps=16*y, yT8=32*y, thT8=16*th; glog=psum/64, logits=psum/256.
"""
import os
import sys
from contextlib import ExitStack

for _p in ("/opt/trn_rl_repo", "/root/.axon_site/_ro/trn_rl_repo"):
    if os.path.isdir(_p) and _p not in sys.path:
        sys.path.insert(0, _p)

import numpy as np
import ml_dtypes

import concourse.bass as bass
import concourse.tile as tile
from concourse import bacc, mybir
from concourse.masks import make_identity
from concourse.bass_utils import run_bass_kernel_spmd

F32 = mybir.dt.float32
BF16 = mybir.dt.bfloat16
FP8 = mybir.dt.float8e4
BF = ml_dtypes.bfloat16
F8 = ml_dtypes.float8_e4m3
AF = mybir.ActivationFunctionType
OP = mybir.AluOpType
AX = mybir.AxisListType
PM = mybir.MatmulPerfMode

NCORES = 8
B, C, T, H, E, EH, TH = 512, 30, 3, 512, 6, 512, 512
BSH = B // NCORES          # 64 samples per core
R = BSH * C                # 1920 rows per core
NRT = R // 128             # 15 row tiles
KC = H // 128              # 4 feature chunks
GE = T * E                 # 18
RS = [(0, 512), (512, 1024), (1024, 1536), (1536, R)]   # row slices (<=512)
RG_RT = [(0, 4), (4, 8), (8, 12), (12, 15)]             # row tiles per group
LOSS_COEF = 0.01

WA_W = 3072                # packed wa columns per k-chunk: xT | fc1 | fc2 | wg | pad
# (padded so the DoubleRow pair stride is 0 mod 4: dual-fp8 ldweights
#  requires aligned pair strides per the s3_lw_dual_fp8 ISA check)

# engine split knobs: weighted rotation across (ACT, DVE, Pool)
# NOTE: GPSIMD/Pool cannot access PSUM on real hardware (BIR verifier),
# so evacuations rotate over ACT/DVE only; Pool absorbs the SBUF->SBUF
# diag(g) builds.
KNOBS = {
    "evac_w": (1, 1, 0),   # PSUM evacuations (pool weight must stay 0)
    "diag_w": (0, 0, 1),   # diag(g) builds
}

_CACHED = {}


class EngRR:
    """Weighted round-robin engine chooser."""

    def __init__(self, nc, weights):
        self.nc = nc
        self.pattern = []
        for eng, w in zip(("act", "dve", "pool"), weights):
            self.pattern += [eng] * w
        self.i = 0

    def pick(self):
        eng = self.pattern[self.i % len(self.pattern)]
        self.i += 1
        return eng

    def copy(self, dst, src, scale=1.0, relu=False, bias=None):
        """dst = [relu?]((src * scale) + bias); bias is per-partition AP."""
        nc = self.nc
        if bias is not None:
            func = AF.Relu if relu else AF.Identity
            nc.scalar.activation(dst, src, func, bias=bias, scale=scale)
            return
        eng = self.pick()
        if eng == "act":
            nc.scalar.activation(dst, src, AF.Relu if relu else AF.Copy,
                                 scale=scale)
        elif eng == "dve":
            if relu:
                nc.vector.tensor_scalar(dst, src, scale, 0.0, OP.mult, OP.max)
            elif scale != 1.0:
                nc.vector.tensor_scalar(dst, src, scale, None, OP.mult)
            else:
                nc.vector.tensor_copy(dst, src)
        else:
            if relu:
                nc.gpsimd.tensor_scalar(dst, src, scale, 0.0, OP.mult, OP.max)
            elif scale != 1.0:
                nc.gpsimd.tensor_scalar(dst, src, scale, None, OP.mult)
            else:
                nc.gpsimd.tensor_copy(dst, src)

    def scale_by(self, dst, src, g_ap):
        """dst = src * g (per-partition scalar AP); used for diag builds."""
        nc = self.nc
        eng = self.pick()
        if eng == "act":
            nc.scalar.activation(dst, src, AF.Copy, scale=g_ap)
        elif eng == "dve":
            nc.vector.tensor_scalar(dst, src, g_ap, None, OP.mult)
        else:
            nc.gpsimd.tensor_scalar(dst, src, g_ap, None, OP.mult)


def build_nc(zero_bias: bool):
    nc = bacc.Bacc(None, target_bir_lowering=False, debug=False)

    wa_d = nc.dram_tensor("wa", [128, KC, WA_W], FP8, kind="ExternalInput")
    wb_d = nc.dram_tensor("wb", [128, E, 2, KC, 512], FP8, kind="ExternalInput")
    wc_d = nc.dram_tensor("wc", [128, T, KC, TH], FP8, kind="ExternalInput")
    tw2_d = nc.dram_tensor("tw2", [128, KC, T, T], FP8, kind="ExternalInput")
    scores_d = nc.dram_tensor("scores", [BSH, T, C], F32, kind="ExternalInput")
    srm_d = nc.dram_tensor("srm", [128, NRT, T], F32, kind="ExternalInput")
    sel_d = nc.dram_tensor("sel", [NRT, 128, BSH], F32, kind="ExternalInput")
    selt_d = nc.dram_tensor("selt", [NRT, BSH, 128], F32, kind="ExternalInput")
    if not zero_bias:
        fc1b_d = nc.dram_tensor("fc1b", [128, KC], F32, kind="ExternalInput")
        fc2b_d = nc.dram_tensor("fc2b", [128, KC], F32, kind="ExternalInput")
        eb1_d = nc.dram_tensor("eb1", [E, 128, KC], F32, kind="ExternalInput")
        eb2_d = nc.dram_tensor("eb2", [E, 1, H], BF16, kind="ExternalInput")
        tb1_d = nc.dram_tensor("tb1", [T, 128, KC], F32, kind="ExternalInput")
        tb2_d = nc.dram_tensor("tb2", [128, 1, T], F32, kind="ExternalInput")
    loss_d = nc.dram_tensor("loss", [BSH, 1], F32, kind="ExternalOutput")

    with tile.TileContext(nc, pool_alloc_mode="queue") as tc, ExitStack() as ctx:
        rr = EngRR(nc, KNOBS["evac_w"])
        yrr = rr
        trr = rr
        drr = EngRR(nc, KNOBS["diag_w"])

        perm = ctx.enter_context(tc.tile_pool(name="perm", bufs=1))
        psA = ctx.enter_context(tc.tile_pool(name="psA", bufs=4, space="PSUM"))

        ident8 = perm.tile([128, 128], FP8)
        make_identity(nc, ident8)
        ident_f = perm.tile([128, 128], F32)
        make_identity(nc, ident_f)
        if not zero_bias:
            ones_bf = perm.tile([1, 128], BF16)
            nc.vector.memset(ones_bf, 1.0)

        warm = perm.tile([128, 1], F32)
        nc.scalar.activation(warm, ident_f[:, 0:1], AF.Exp)
        nc.scalar.activation(warm, ident_f[:, 0:1], AF.Abs)
        nc.scalar.activation(warm, ident_f[:, 0:1], AF.Ln, bias=1.0)

        # ---- input loads (few big DMAs) ----
        wa = perm.tile([128, KC, WA_W], FP8)
        nc.sync.dma_start(wa, wa_d[:, :, :])
        wb = perm.tile([128, E, 2, KC, 512], FP8)
        nc.sync.dma_start(wb, wb_d[:, :, :, :, :])
        wc = perm.tile([128, T, KC, TH], FP8)
        nc.sync.dma_start(wc, wc_d[:, :, :, :])
        tw2 = perm.tile([128, KC, T, T], FP8)
        nc.sync.dma_start(tw2, tw2_d[:, :, :, :])
        scores_sb = perm.tile([BSH, T, C], F32)
        nc.sync.dma_start(scores_sb, scores_d[:, :, :])
        srm_sb = perm.tile([128, NRT, T], F32)
        nc.sync.dma_start(srm_sb, srm_d[:, :, :])
        sel_sb = perm.tile([128, NRT, BSH], F32)
        nc.sync.dma_start(sel_sb, sel_d.rearrange("rt p b -> p rt b"))
        selt_sb = perm.tile([BSH, NRT, 128], F32)
        nc.sync.dma_start(selt_sb, selt_d.rearrange("rt b p -> b rt p"))
        if not zero_bias:
            fc1b = perm.tile([128, KC], F32)
            nc.sync.dma_start(fc1b, fc1b_d[:, :])
            fc2b = perm.tile([128, KC], F32)
            nc.sync.dma_start(fc2b, fc2b_d[:, :])
            eb1 = perm.tile([128, E, KC], F32)
            nc.sync.dma_start(eb1, eb1_d.rearrange("e p k -> p e k"))
            eb2 = perm.tile([E, 1, H], BF16)
            nc.sync.dma_start(eb2, eb2_d[:, :, :])
            tb1 = perm.tile([128, T, KC], F32)
            nc.sync.dma_start(tb1, tb1_d.rearrange("t p k -> p t k"))
            tb2_sb = perm.tile([128, 1, T], F32)
            nc.sync.dma_start(tb2_sb, tb2_d[:, :, :])

        RP = 2048  # padded row pitch so paired evacs stay in-bounds
        h1T8 = perm.tile([128, KC, RP], FP8)
        hT8 = perm.tile([128, KC, RP], FP8)
        glog = perm.tile([128, NRT, T, E], F32)
        gates = perm.tile([128, NRT, T, E], F32)
        yT8 = perm.tile([128, KC, T, R], FP8)
        thT8 = [perm.tile([128, KC, R], FP8, name=f"thT{t}") for t in range(T)]
        logits_sb = perm.tile([128, NRT, T], F32)
        labels_rm = perm.tile([128, NRT, T], F32)
        t1_ = perm.tile([128, NRT, T], F32)
        bce_a = perm.tile([128, NRT * T], F32)
        bce_b = perm.tile([128, NRT * T], F32)
        auxs = perm.tile([BSH, 1], F32)

        with tc.tile_pool(name="psS", bufs=1, space="PSUM") as psS:
            # ---------------- phase 1: shared bottom ----------------
            def fc_layer(wofs, dst, rhs_of, scale, relu, bias_t):
                for (r0, r1) in RS:
                    for mc in range(KC):
                        ps = psA.tile([128, 512], F32, name="acc", tag="acc")
                        pp = ps[:, 0:r1 - r0]
                        for j in range(2):
                            nc.tensor.matmul(
                                pp, wa[:, 2 * j:2 * j + 2,
                                       wofs + mc * 128:wofs + (mc + 1) * 128],
                                rhs_of(j, r0, r1),
                                start=(j == 0), stop=(j == 1),
                                perf_mode=PM.DoubleRow)
                        bias = None if bias_t is None else bias_t[:, mc:mc + 1]
                        rr.copy(dst[:, mc, r0:r1], pp,
                                scale=scale, relu=relu, bias=bias)

            fc_layer(R, h1T8, lambda j, r0, r1: wa[:, 2 * j:2 * j + 2, r0:r1],
                     0.25, True, None if zero_bias else fc1b)
            fc_layer(R + H, hT8,
                     lambda j, r0, r1: h1T8[:, 2 * j:2 * j + 2, r0:r1],
                     1.0 / 16, False, None if zero_bias else fc2b)

            # ---------------- phase 2: gate logits ----------------
            gflat = glog.rearrange("p rt t e -> p (rt t e)")
            for rt in range(NRT):
                ps = psS.tile([128, GE], F32, name="gacc", tag="gacc", bufs=2)
                for j in range(2):
                    nc.tensor.matmul(
                        ps, hT8[:, 2 * j:2 * j + 2, rt * 128:(rt + 1) * 128],
                        wa[:, 2 * j:2 * j + 2, R + 2 * H:R + 2 * H + GE],
                        start=(j == 0), stop=(j == 1), perf_mode=PM.DoubleRow)
                nc.vector.tensor_scalar(
                    gflat[:, rt * GE:(rt + 1) * GE], ps, 1.0 / 64, None,
                    OP.mult)

            # ---------------- phase 3: top-3-of-6 masked softmax ----------------
            NG = NRT * T
            v = glog.rearrange("p rt t e -> p (rt t) e")
            with tc.tile_pool(name="gtmp", bufs=1) as gtmp:
                neginf = gtmp.tile([128, NG, E], F32)
                nc.vector.memset(neginf, -1e30)
                m1 = gtmp.tile([128, NG, 1], F32)
                nc.vector.tensor_reduce(m1, v, AX.X, OP.max)
                m1b = m1.broadcast_to([128, NG, E])
                mask = gtmp.tile([128, NG, E], mybir.dt.uint8)
                nc.vector.tensor_tensor(mask, v, m1b, OP.is_ge)
                v2 = gtmp.tile([128, NG, E], F32)
                nc.vector.select(v2, mask, neginf, v)
                m2 = gtmp.tile([128, NG, 1], F32)
                nc.vector.tensor_reduce(m2, v2, AX.X, OP.max)
                mask2 = gtmp.tile([128, NG, E], mybir.dt.uint8)
                nc.vector.tensor_tensor(mask2, v2,
                                        m2.broadcast_to([128, NG, E]), OP.is_ge)
                v3 = gtmp.tile([128, NG, E], F32)
                nc.vector.select(v3, mask2, neginf, v2)
                m3 = gtmp.tile([128, NG, 1], F32)
                nc.vector.tensor_reduce(m3, v3, AX.X, OP.max)
                keep = gtmp.tile([128, NG, E], F32)
                nc.vector.tensor_tensor(keep, v,
                                        m3.broadcast_to([128, NG, E]), OP.is_ge)
                vs = gtmp.tile([128, NG, E], F32)
                nc.vector.tensor_tensor(vs, v, m1b, OP.subtract)
                ex = gtmp.tile([128, NG, E], F32)
                nc.scalar.activation(ex, vs, AF.Exp)
                ek = gtmp.tile([128, NG, E], F32)
                nc.vector.tensor_tensor(ek, ex, keep, OP.mult)
                ssum = gtmp.tile([128, NG, 1], F32)
                nc.vector.tensor_reduce(ssum, ek, AX.X, OP.add)
                rsum = gtmp.tile([128, NG, 1], F32)
                nc.vector.reciprocal(rsum, ssum)
                gv = gates.rearrange("p rt t e -> p (rt t) e")
                nc.vector.tensor_tensor(gv, ek,
                                        rsum.broadcast_to([128, NG, E]), OP.mult)

            # ---- aux loss: imp[b, (t e)] via selector matmuls ----
            g2 = gates.rearrange("p rt t e -> p rt (t e)")
            ip = psS.tile([BSH, GE], F32, name="iacc", tag="gacc", bufs=2)
            for rt in range(NRT):
                nc.tensor.matmul(ip, sel_sb[:, rt, :], g2[:, rt, :],
                                 start=(rt == 0), stop=(rt == NRT - 1))
            impT = perm.tile([BSH, GE], F32)
            nc.vector.tensor_copy(impT, ip)
            impTv = impT.rearrange("b (t e) -> b t e", e=E)
            for t in range(T):
                st = perm.tile([BSH, 6], F32, name=f"bnst{t}")
                nc.vector.bn_stats(st, impTv[:, t, :])
                mv = perm.tile([BSH, 2], F32, name=f"bnmv{t}")
                nc.vector.bn_aggr(mv, st)
                msq = perm.tile([BSH, 1], F32, name=f"msq{t}")
                nc.vector.tensor_tensor(msq, mv[:, 0:1], mv[:, 0:1], OP.mult)
                nc.vector.tensor_scalar(msq, msq, 1e-10, None, OP.add)
                rec = perm.tile([BSH, 1], F32, name=f"rec{t}")
                nc.vector.reciprocal(rec, msq)
                cv2 = perm.tile([BSH, 1], F32, name=f"cv2{t}")
                nc.vector.tensor_tensor(cv2, mv[:, 1:2], rec, OP.mult)
                if t == 0:
                    nc.vector.tensor_copy(auxs, cv2)
                else:
                    nc.vector.tensor_tensor(auxs, auxs, cv2, OP.add)

            # ---- labels (row-major): smax broadcast via selector matmuls ----
            smax = perm.tile([BSH, T], F32)
            smax3 = perm.tile([BSH, T, 1], F32)
            nc.vector.tensor_reduce(smax3, scores_sb, AX.X, OP.max)
            nc.vector.tensor_copy(smax, smax3.rearrange("b t one -> b (t one)"))
            smax_bc = perm.tile([128, NRT, T], F32)
            for rt in range(NRT):
                pb = psS.tile([128, T], F32, name="smb", tag="small", bufs=2)
                nc.tensor.matmul(pb, selt_sb[:, rt, :], smax,
                                 start=True, stop=True)
                nc.vector.tensor_copy(smax_bc[:, rt, :], pb)
            nc.vector.tensor_tensor(labels_rm, srm_sb, smax_bc, OP.is_equal)

        # ---------------- phase 4: experts + fused gated combine ----------------
        # Software-pipelined over row groups: A(g)=experts, B(g)=fused combine,
        # T(g)=towers, emitted A0 A1 B0 A2 B1 T0 A3 B2 T1 B3 T2 T3 so each
        # stage's waits land on work issued ~a full stage earlier.
        with tc.tile_pool(name="exp", bufs=1) as exp, \
             tc.tile_pool(name="psY", bufs=1, space="PSUM") as psY:
            eo8s = {}
            diags = {}

            def stage_A(gi):
                (r0, r1), (t0, t1) = RS[gi], RG_RT[gi]
                rgw = r1 - r0
                nrg = t1 - t0
                npair = (nrg + 1) // 2
                eo8s[gi] = [exp.tile([128, E, 2, H], FP8, name=f"eo{gi}_{pi}",
                                     tag="eo8", bufs=4)
                            for pi in range(npair)]
                for rtl in range(nrg):
                    rt = t0 + rtl
                    diag = exp.tile([128, E, T * 128], FP8,
                                    name=f"diag{rt}", tag="diag", bufs=8)
                    diags[rt] = diag
                    for t in range(T):
                        for e in range(E):
                            drr.scale_by(diag[:, e, t * 128:(t + 1) * 128],
                                         ident8, gates[:, rt, t, e:e + 1])
                for e in range(E):
                    ehs = exp.tile([128, KC, 512], FP8, name="ehs", tag="ehs",
                                   bufs=3)
                    for mc in range(KC):
                        ps = psA.tile([128, 512], F32, name="eacc", tag="acc")
                        pp = ps[:, 0:rgw]
                        for j in range(2):
                            nc.tensor.matmul(
                                pp,
                                wb[:, e, 0, 2 * j:2 * j + 2,
                                   mc * 128:(mc + 1) * 128],
                                hT8[:, 2 * j:2 * j + 2, r0:r1],
                                start=(j == 0), stop=(j == 1),
                                perf_mode=PM.DoubleRow)
                        rr.copy(ehs[:, mc, 0:rgw], pp, scale=0.125, relu=True,
                                bias=None if zero_bias else eb1[:, e, mc:mc + 1])
                    for rtl in range(nrg):
                        ps = psA.tile([128, 512], F32, name="oacc", tag="acc")
                        for j in range(2):
                            nc.tensor.matmul(
                                ps, ehs[:, 2 * j:2 * j + 2,
                                        rtl * 128:(rtl + 1) * 128],
                                wb[:, e, 1, 2 * j:2 * j + 2, :],
                                start=(j == 0), stop=(j == 1) and zero_bias,
                                perf_mode=PM.DoubleRow)
                        if not zero_bias:
                            nc.tensor.matmul(ps, ones_bf, eb2[e],
                                             start=False, stop=True)
                        rr.copy(eo8s[gi][rtl // 2][:, e, rtl % 2, :], ps,
                                scale=0.125)

            def stage_B(gi):
                (r0, r1), (t0, t1) = RS[gi], RG_RT[gi]
                for rtl in range(t1 - t0):
                    rt = t0 + rtl
                    diag = diags.pop(rt)
                    eo = eo8s[gi][rtl // 2]
                    hh = rtl % 2
                    for mc in range(KC):
                        yps = psY.tile([128, T, 128], F32, name="yps",
                                       tag="yps", bufs=4)
                        for i in range(E // 2):
                            nc.tensor.matmul(
                                yps.rearrange("p t r -> p (t r)"),
                                eo[:, 2 * i:2 * i + 2, hh,
                                   mc * 128:(mc + 1) * 128],
                                diag[:, 2 * i:2 * i + 2, :],
                                start=(i == 0), stop=(i == E // 2 - 1),
                                perf_mode=PM.DoubleRow)
                        yrr.copy(yT8[:, mc, :, rt * 128:(rt + 1) * 128],
                                 yps, scale=2.0)

            def stage_T(gi):
                (r0, r1) = RS[gi]
                rgw = r1 - r0
                for t in range(T):
                    for mc in range(KC):
                        ps = psA.tile([128, 512], F32, name="tacc", tag="acc")
                        pp = ps[:, 0:rgw]
                        for j in range(2):
                            nc.tensor.matmul(
                                pp, wc[:, t, 2 * j:2 * j + 2,
                                       mc * 128:(mc + 1) * 128],
                                yT8[:, 2 * j:2 * j + 2, t, r0:r1],
                                start=(j == 0), stop=(j == 1),
                                perf_mode=PM.DoubleRow)
                        trr.copy(thT8[t][:, mc, r0:r1], pp, scale=1.0 / 32,
                                 relu=True,
                                 bias=None if zero_bias else tb1[:, t, mc:mc + 1])
            stage_A(0)
            stage_A(1)
            stage_B(0)
            stage_A(2)
            stage_B(1)
            stage_T(0)
            stage_A(3)
            stage_B(2)
            stage_T(1)
            stage_B(3)
            stage_T(2)
            stage_T(3)

        # ---------------- phase 5: logits; phase 6: BCE + loss ----------------
        with tc.tile_pool(name="psL", bufs=1, space="PSUM") as psL:
            for rt in range(NRT):
                pl = psL.tile([128, T], F32, name="lg", tag="lg", bufs=2)
                n = 0
                for t in range(T):
                    for k in range(KC):
                        n += 1
                        nc.tensor.matmul(
                            pl, thT8[t][:, k, rt * 128:(rt + 1) * 128],
                            tw2[:, k, t, :],
                            start=(n == 1), stop=(n == KC * T))
                nc.vector.tensor_scalar(logits_sb[:, rt, :], pl, 1.0 / 256,
                                        None, OP.mult)
            lg = logits_sb
            if not zero_bias:
                nc.vector.tensor_tensor(lg, lg,
                                        tb2_sb.broadcast_to([128, NRT, T]),
                                        OP.add)
            t2_ = perm.tile([128, NRT, T], F32)
            nc.vector.tensor_scalar(t1_, lg, 0.0, None, OP.max)
            nc.vector.tensor_tensor(t2_, lg, labels_rm, OP.mult)
            nc.vector.tensor_tensor(t1_, t1_, t2_, OP.subtract)
            absl = perm.tile([128, NRT, T], F32)
            nc.scalar.activation(absl, lg, AF.Abs)
            nc.scalar.activation(absl, absl, AF.Exp, scale=-1.0)
            nc.scalar.activation(absl, absl, AF.Ln, bias=1.0)
            nc.vector.tensor_tensor(t1_, t1_, absl, OP.add)
            bs = perm.tile([128, NRT], F32)
            nc.vector.tensor_reduce(bs, t1_, AX.X, OP.add)
            pb = psL.tile([BSH, 1], F32, name="bsum", tag="lg", bufs=2)
            for rt in range(NRT):
                nc.tensor.matmul(pb, sel_sb[:, rt, :], bs[:, rt:rt + 1],
                                 start=(rt == 0), stop=(rt == NRT - 1))
            tsum = perm.tile([BSH, 1], F32)
            nc.vector.tensor_copy(tsum, pb)

            loss_sb = perm.tile([BSH, 1], F32)
            nc.vector.tensor_scalar(loss_sb, tsum, 1.0 / (T * C), None, OP.mult)
            auxf = perm.tile([BSH, 1], F32)
            nc.vector.tensor_scalar(auxf, auxs, LOSS_COEF, None, OP.mult)
            nc.vector.tensor_tensor(loss_sb, loss_sb, auxf, OP.add)
            nc.sync.dma_start(loss_d[:, :], loss_sb)

    nc.compile()
    return nc


def get_nc(zero_bias=True):
    key = (zero_bias, tuple(sorted((k, tuple(v)) for k, v in KNOBS.items())))
    if key not in _CACHED:
        _CACHED[key] = build_nc(zero_bias)
    return _CACHED[key]


_SEL_CACHE = None


def _sel_mats():
    """0/1 selector matrices mapping rows r=rt*128+p to samples b=r//30."""
    global _SEL_CACHE
    if _SEL_CACHE is None:
        sel = np.zeros((NRT, 128, BSH), np.float32)
        for rt in range(NRT):
            for p in range(128):
                b = (rt * 128 + p) // C
                sel[rt, p, b] = 1.0
        selt = np.ascontiguousarray(sel.transpose(0, 2, 1))
        _SEL_CACHE = (sel, selt)
    return _SEL_CACHE


def _fm8(w, scale=16.0):
    """[H, M] f32 -> [128, KC, M] fp8 feature-major (partition, k-chunk)."""
    Hd, M = w.shape
    return np.ascontiguousarray(
        (w * scale).reshape(KC, 128, M).transpose(1, 0, 2)).astype(F8)


def host_prep(inputs):
    """Shard + cast + pack the full inputs into 8 per-core in_maps."""
    x = np.asarray(inputs["candidate_cls_embed"], np.float32)
    scores = np.asarray(inputs["scores"], np.float32)
    fc1_b = np.asarray(inputs["fc1_b"], np.float32)
    fc2_b = np.asarray(inputs["fc2_b"], np.float32)
    expert_b1 = np.asarray(inputs["expert_b1"], np.float32)
    expert_b2 = np.asarray(inputs["expert_b2"], np.float32)
    tower_b1 = np.asarray(inputs["tower_b1"], np.float32)
    tower_b2 = np.asarray(inputs["tower_b2"], np.float32)
    fc1_w = np.asarray(inputs["fc1_w"], np.float32)
    fc2_w = np.asarray(inputs["fc2_w"], np.float32)
    w_gate = np.asarray(inputs["w_gate"], np.float32)
    expert_w1 = np.asarray(inputs["expert_w1"], np.float32)
    expert_w2 = np.asarray(inputs["expert_w2"], np.float32)
    tower_w1 = np.asarray(inputs["tower_w1"], np.float32)
    tower_w2 = np.asarray(inputs["tower_w2"], np.float32)

    zero_bias = not (fc1_b.any() or fc2_b.any() or expert_b1.any()
                     or expert_b2.any() or tower_b1.any() or tower_b2.any())

    wa_shared = np.zeros((128, KC, WA_W), F8)
    wa_shared[:, :, R:R + H] = _fm8(fc1_w)
    wa_shared[:, :, R + H:R + 2 * H] = _fm8(fc2_w)
    wg2d = np.ascontiguousarray(w_gate.transpose(1, 0, 2).reshape(H, GE))
    wa_shared[:, :, R + 2 * H:R + 2 * H + GE] = _fm8(wg2d)

    wb = np.zeros((128, E, 2, KC, 512), F8)
    for e in range(E):
        wb[:, e, 0] = _fm8(expert_w1[e])
        wb[:, e, 1] = _fm8(expert_w2[e])

    wc = np.zeros((128, T, KC, TH), F8)
    for t in range(T):
        wc[:, t] = _fm8(tower_w1[t])
    # tw2 one-hot padded: tw2p[p, k, t, u] = 16*tw2[t, k*128+p] * (u == t)
    tw2p = np.zeros((128, KC, T, T), F8)
    tw2f = (16.0 * tower_w2).reshape(T, KC, 128)
    for t in range(T):
        tw2p[:, :, t, t] = tw2f[t].T.astype(F8)

    shared = {
        "wb": wb,
        "wc": wc,
        "tw2": tw2p,
        "sel": _sel_mats()[0],
        "selt": _sel_mats()[1],
    }
    if not zero_bias:
        shared.update({
            "fc1b": np.ascontiguousarray(4.0 * fc1_b.reshape(KC, 128).T),
            "fc2b": np.ascontiguousarray(4.0 * fc2_b.reshape(KC, 128).T),
            "eb1": np.ascontiguousarray(
                8.0 * expert_b1.reshape(E, KC, 128).transpose(0, 2, 1)),
            "eb2": (8.0 * expert_b2).astype(BF).reshape(E, 1, H),
            "tb1": np.ascontiguousarray(
                16.0 * tower_b1.reshape(T, KC, 128).transpose(0, 2, 1)),
            "tb2": np.ascontiguousarray(
                np.broadcast_to(tower_b2[None, None, :],
                                (128, 1, T)).astype(np.float32)),
        })

    in_maps = []
    for ci in range(NCORES):
        xs = x[ci * BSH:(ci + 1) * BSH].reshape(R, H)
        wa = wa_shared.copy()
        wa[:, :, 0:R] = np.ascontiguousarray(
            xs.T.reshape(KC, 128, R).transpose(1, 0, 2)).astype(F8)
        m = dict(shared)
        m["wa"] = wa
        sc = np.ascontiguousarray(scores[ci * BSH:(ci + 1) * BSH])
        m["scores"] = sc
        srm = sc.transpose(0, 2, 1).reshape(NRT, 128, T).transpose(1, 0, 2)
        m["srm"] = np.ascontiguousarray(srm)
        in_maps.append(m)
    return in_maps, zero_bias


def kernel(**inputs) -> np.ndarray:
    in_maps, zero_bias = host_prep(inputs)
    nc = get_nc(zero_bias)
    res = run_bass_kernel_spmd(nc, in_maps, list(range(NCORES)))
    losses = np.concatenate([res.results[i]["loss"].reshape(-1)
                             for i in range(NCORES)])
    return np.float32(losses.mean(dtype=np.float64))
